# revision 56
# baseline (speedup 1.0000x reference)
"""Trainium2 Bass kernel: LayerNorm -> QKV -> linear (elu+1) attention -> proj.

Data-parallel over batch: 8 batch elements, one per NeuronCore. All matmuls
in bf16 (fp32 accumulation in PSUM); LayerNorm statistics in fp32; the
projection bias is applied in fp32.

Self-contained: hardcodes shapes from the problem spec.
"""

import numpy as np
import ml_dtypes

from concourse import bass, bacc, tile, mybir
from concourse.bass import ts, ds
from concourse.bass_utils import run_bass_kernel_spmd

F32 = mybir.dt.float32
F32R = mybir.dt.float32r
BF16 = mybir.dt.bfloat16
AF = mybir.ActivationFunctionType
ALU = mybir.AluOpType

# Problem shapes
N = 4096          # tokens per batch element
D = 768           # model dim
H = 12            # heads
HD = 64           # head dim
E3 = 3 * D        # qkv width
P = 128
KT = D // P       # 6 d-tiles
NT = N // P       # 32 token tiles
CH = 8            # token chunks of 512
TPC = NT // CH    # 4 token tiles per chunk
CW = N // CH      # 512 chunk width
LN_EPS = 1e-5
EPS = 1e-6

N_CORES = 8
LDW_SKIP = True


def _build(with_qkv_bias: bool, dbg: bool = False):
    """Build the single-core program (SPMD: same NEFF on all 8 cores)."""
    nc = bacc.Bacc("TRN2", target_bir_lowering=False, debug=False,
                   num_devices=N_CORES)

    x_d = nc.dram_tensor("x", [N, D], BF16, kind="ExternalInput").ap()
    wqkvT_d = nc.dram_tensor("wqkvT", [D, E3], BF16, kind="ExternalInput").ap()
    wprojT_d = nc.dram_tensor("wprojT", [D, D], BF16, kind="ExternalInput").ap()
    bpack_d = nc.dram_tensor("bpack", [1, P + D], F32, kind="ExternalInput").ap()
    if with_qkv_bias:
        cqkv_d = nc.dram_tensor("cqkv", [1, E3], F32, kind="ExternalInput").ap()
    out_d = nc.dram_tensor("out", [N, D], BF16, kind="ExternalOutput").ap()

    from contextlib import ExitStack
    with tile.TileContext(nc) as tc, ExitStack() as stk:
        _kernel(tc, stk, nc, x_d, wqkvT_d, wprojT_d, bpack_d,
                cqkv_d if with_qkv_bias else None, out_d, dbg)

    nc.compile()
    return nc


def _kernel(tc, stk, nc, x_d, wqkvT_d, wprojT_d, bpack_d, cqkv_d, out_d,
            dbg=False):
    def dump(name, tl, shape, dtype):
        if not dbg:
            return
        d = nc.dram_tensor("dbg_" + name, shape, dtype, kind="ExternalOutput").ap()
        nc.sync.dma_start(d, tl)

    from contextlib import ExitStack
    consts = stk.enter_context(tc.tile_pool(name="consts", bufs=1))
    stk1 = stk.enter_context(ExitStack())
    ppersist = stk1.enter_context(tc.tile_pool(name="ppersist", bufs=1, space="PSUM"))

    # DMA queue split (HWDGE queues are in-order, so DMAs that wait on
    # compute must not sit ahead of dep-free ones):
    #   SP queue:  x tiles (issued 2 chunks ahead of use, so they always sit
    #              in front of the waiting bounce/transposes), the
    #              LN-dependent bounce + transposes, phase-2 zb/zr.
    #   ACT queue: weights (q columns first; w_proj deferred), out stores.
    x_prefetch = {}
    xTp = stk.enter_context(tc.tile_pool(name="xT", bufs=8))
    xpool_early = stk1.enter_context(tc.tile_pool(name="x", bufs=14))

    def load_x_chunk(c0):
        for t in range(c0 * TPC, (c0 + 1) * TPC):
            xt = xpool_early.tile([P, D], BF16)
            nc.sync.dma_start(xt[:], x_d[ts(t, P), :])
            x_prefetch[t] = xt

    wqkvT = consts.tile([P, KT, E3], BF16)
    wprojT = consts.tile([P, KT, D], BF16)
    wq_r = wqkvT_d.rearrange("(kt p) e -> p kt e", p=P)
    wp_r = wprojT_d.rearrange("(kt p) e -> p kt e", p=P)
    bproj_row = consts.tile([1, D], F32)
    nc.scalar.dma_start(bproj_row[:], bpack_d[:, P:P + D])
    load_x_chunk(0)
    load_x_chunk(1)
    for kt in range(KT):
        nc.scalar.dma_start(wqkvT[:, kt, 0:D], wq_r[:, kt, 0:D])
    for kt in range(KT):
        nc.scalar.dma_start(wqkvT[:, kt, D:E3], wq_r[:, kt, D:E3])

    ones_row = consts.tile([1, P], F32)
    nc.vector.memset(ones_row[:], 1.0)
    bias_sb = consts.tile([P, D], F32)

    # zero-row for psum-bank init matmuls
    zrow = consts.tile([1, 512], BF16)
    nc.vector.memset(zrow[:], 0.0)
    ones_bf = consts.tile([1, P], BF16)
    nc.vector.memset(ones_bf[:], 1.0)

    # --- kv accumulator ---
    # pair p = h//2 -> cols [65p, 65p+65), head parity s=h%2 -> partitions
    # [64s, 64s+64). col 64 of each head block = k_sum.
    kv_ps = ppersist.tile([P, 6 * 65], F32)
    # Init the whole kv bank with one start=True matmul writing zeros: sets
    # every has_written bit so the 12 interleaved accumulation chains below
    # can all run with start=False. (start=True clears the *bank's* bits, so
    # per-chain start flags would clobber each other.)
    nc.tensor.matmul(kv_ps[:], ones_bf[:], zrow[:, 0:6 * 65], start=True,
                     stop=False, skip_group_check=True)

    xpool = xpool_early
    stat = stk1.enter_context(tc.tile_pool(name="stat", bufs=12))
    xhatp = stk1.enter_context(tc.tile_pool(name="xhat", bufs=4))
    kvps = stk1.enter_context(tc.tile_pool(name="kvps", bufs=2, space="PSUM"))
    qpsp = stk1.enter_context(tc.tile_pool(name="qpsp", bufs=1, space="PSUM"))
    evac = stk1.enter_context(tc.tile_pool(name="evac", bufs=4))

    qT_all = consts.tile([P, KT, N], BF16)
    dramp = stk.enter_context(tc.tile_pool(name="dram", bufs=5, space="DRAM"))

    # ============ PHASE 1: LN, transpose, k/v, kv accumulation ============
    # Processed in "pieces" of ntt token-tiles. Chunk 0 runs as two
    # 256-token halves so its LN -> bounce -> transpose fill latency is
    # halved and the PE gets its first matmuls ~15us earlier; later chunks
    # run full-width (the pipeline hides their latency).
    from contextlib import nullcontext

    def process_piece(c, tt0, ntt, q_first):
        W = ntt * P
        base = c * CW + tt0 * P
        xts = []
        mv_all = stat.tile([P, TPC, 2], F32, tag="mv")
        # For the first pipeline-filling piece, rank the LN -> xhat chain
        # ahead of neighboring pieces' stats on the in-order DVE queue.
        prio = tc.high_priority() if (c == 0 and tt0 == 0) else nullcontext()
        with prio:
            for j in range(ntt):
                xt = x_prefetch.pop(c * TPC + tt0 + j)
                xts.append(xt)
                # LayerNorm stats (fp32)
                st6 = stat.tile([P, 2, 6], F32)
                nc.vector.bn_stats(st6[:, 0], xt[:, 0:D // 2])
                nc.vector.bn_stats(st6[:, 1], xt[:, D // 2:D])
                nc.vector.bn_aggr(mv_all[:, j], st6[:])
            # batched rstd = rsqrt(var+eps): bit-trick seed + 1 Newton step
            # (seed rel err ~3.4% -> ~0.2% after one step; xhat is bf16)
            I32 = mybir.dt.int32
            veps = stat.tile([P, TPC], F32)
            nc.vector.tensor_scalar_add(veps[:, 0:ntt], mv_all[:, 0:ntt, 1],
                                        LN_EPS)
            t1 = stat.tile([P, TPC], I32, tag="rs_t1")
            nc.vector.tensor_scalar(t1[:, 0:ntt],
                                    veps[:, 0:ntt].bitcast(I32), 1, None,
                                    op0=ALU.arith_shift_right)
            rstd = stat.tile([P, TPC], F32)
            nc.vector.tensor_scalar(rstd[:, 0:ntt].bitcast(I32), t1[:, 0:ntt],
                                    -1, 0x5F3759DF, op0=ALU.mult, op1=ALU.add)
            a = stat.tile([P, TPC], F32, tag="rs_a")
            nc.vector.tensor_tensor(a[:, 0:ntt], rstd[:, 0:ntt], rstd[:, 0:ntt],
                                    ALU.mult)
            nc.vector.tensor_tensor(a[:, 0:ntt], a[:, 0:ntt], veps[:, 0:ntt],
                                    ALU.mult)
            nc.vector.tensor_scalar(a[:, 0:ntt], a[:, 0:ntt], -0.5, 1.5,
                                    op0=ALU.mult, op1=ALU.add)
            nc.vector.tensor_tensor(rstd[:, 0:ntt], rstd[:, 0:ntt], a[:, 0:ntt],
                                    ALU.mult)
            xhat = xhatp.tile([P, TPC, D], BF16)
            xh_dram = dramp.tile([CW, D], BF16)
            for j in range(ntt):
                # xhat = (x - mean) * rstd   -> bf16
                nc.vector.tensor_scalar(xhat[:, j], xts[j][:],
                                        mv_all[:, j, 0:1],
                                        rstd[:, j:j + 1],
                                        op0=ALU.subtract, op1=ALU.mult)
        # single bounce DMA per piece (one writer for the transposes)
        nc.sync.dma_start(xh_dram[0:W].rearrange("(tt p) d -> p tt d", p=P),
                          xhat[:, 0:ntt])

        # transpose the piece: [t, d] -> [d, t] via DRAM->SBUF DMA, batched
        # as two 3-kt transposes (3D out AP) to amortize the ~1.3us
        # descriptor generation per instruction on SP.
        # (all on SP: concurrent xbar transposes on both HWDGE queues
        # produce corrupted output -- verified empirically)
        xT3 = [xTp.tile([P, 3, CW], BF16, tag="xT3",
                        name=f"xT3_{c}_{tt0}_{h}") for h in range(2)]
        for h in range(2):
            nc.sync.dma_start_transpose(out=xT3[h][:, :, 0:W],
                                        in_=xh_dram[0:W, ds(h * 384, 384)])
        xT = [xT3[kt // 3][:, kt % 3, 0:W] for kt in range(KT)]

        # --- q (weight stationary, directly transposed) interleaved with
        # k/v (activation stationary) so PSUM evacuations never stall PE ---
        def q_chain(m):
            q_ps = qpsp.tile([P, 512], F32, tag="qps1")
            for kt in range(KT):
                nc.tensor.matmul(q_ps[:, 0:W], wqkvT[:, kt, ts(m, P)], xT[kt],
                                 start=(kt == 0), stop=(kt == KT - 1))
            # elu1(q) = min(exp(q),1) + relu(q); exp+relu on ACT, fuse on DVE
            et = evac.tile([P, CW], BF16, tag="elu_e")
            nc.scalar.activation(et[:, 0:W], q_ps[:, 0:W], AF.Exp)
            rt = evac.tile([P, CW], BF16, tag="elu_r")
            nc.scalar.activation(rt[:, 0:W], q_ps[:, 0:W], AF.Relu)
            nc.vector.scalar_tensor_tensor(qT_all[:, m, ds(base, W)],
                                           et[:, 0:W], 1.0, rt[:, 0:W],
                                           op0=ALU.min, op1=ALU.add)

        def kv_chain(j):
            t = c * TPC + tt0 + j
            kv3 = kvps.tile([P, 3 * 512], F32, tag="ph1ps")  # cols [768, 2304)
            for kt in range(KT):
                for jj in range(3):
                    mm = nc.tensor.matmul(
                        kv3[:, ts(jj, 512)],
                        xT[kt][:, ts(j, P)],
                        wqkvT[:, kt, ds(D + jj * 512, 512)],
                        start=(kt == 0), stop=(kt == KT - 1))
                    if jj > 0 and LDW_SKIP:
                        mm.ldweights = False  # same stationary as jj-1
            # k = elu1(cols 0:768) = min(exp, 1) + relu
            ek = evac.tile([P, D], BF16, tag="elu_ek")
            nc.scalar.activation(ek[:], kv3[:, 0:D], AF.Exp)
            rk = evac.tile([P, D], BF16, tag="elu_rk")
            nc.vector.tensor_scalar_max(rk[:], kv3[:, 0:D], 0.0)
            ktile = evac.tile([P, D], BF16, tag="ktile")
            nc.vector.scalar_tensor_tensor(ktile[:], ek[:], 1.0, rk[:],
                                           op0=ALU.min, op1=ALU.add)
            # v' = [v_h | 1] per head: [128, 12, 65]
            vtile = evac.tile([P, H, HD + 1], BF16, tag="vtile")
            nc.vector.memset(vtile[:, :, HD:HD + 1], 1.0)
            nc.scalar.activation(
                vtile[:, :, 0:HD],
                kv3[:, D:2 * D].rearrange("p (h e) -> p h e", h=H),
                AF.Copy)
            # kv accumulation: 12 heads, 2 packed per psum column block
            for h in range(H):
                p_, s_ = h // 2, h % 2
                nc.tensor.matmul(
                    kv_ps[ds(64 * s_, 64), ds(65 * p_, 65)],
                    ktile[:, ds(HD * h, HD)],
                    vtile[:, h],
                    start=False, stop=(t == NT - 1),
                    skip_group_check=True,
                    tile_position=(0, 64 * s_))

        # issue order: q0 kv0 q1 kv1 ... then remaining q chains. With
        # q_first (chunk 0, k/v weight columns still in flight) all q
        # chains run before the kv chains.
        if q_first:
            for m in range(KT):
                q_chain(m)
            for j in range(ntt):
                kv_chain(j)
        else:
            for j in range(ntt):
                q_chain(j)
                kv_chain(j)
            for m in range(ntt, KT):
                q_chain(m)

    for c in range(CH):
        if c == 0:
            process_piece(0, 0, 2, True)
            process_piece(0, 2, 2, True)
        else:
            process_piece(c, 0, TPC, False)
        # x refill for chunk c+2 goes AFTER this chunk's bounce/transposes
        # in the SP queue so it cannot delay them
        if c + 2 < CH:
            load_x_chunk(c + 2)
        if c == 4:
            # w_proj is phase-2-only; issue late-ish on the ACT queue
            for kt in range(KT):
                nc.scalar.dma_start(wprojT[:, kt], wp_r[:, kt])

    # ================= PHASE 1.5: kv -> sbuf, Ksel ========================
    kv_sb = consts.tile([P, 6 * 65], BF16)
    nc.scalar.activation(kv_sb[:], kv_ps[:], AF.Copy)
    dump("kv", kv_sb[:], [P, 6 * 65], BF16)
    dump("qTd", qT_all[:], [P, KT, N], BF16)
    ksel = consts.tile([P, KT, H], BF16)
    nc.vector.memset(ksel[:], 0.0)
    for kt in range(KT):
        for s_ in range(2):
            h = 2 * kt + s_
            nc.vector.tensor_copy(
                ksel[ds(64 * s_, 64), kt, h:h + 1],
                kv_sb[ds(64 * s_, 64), ds(65 * kt + 64, 1)])

    stk1.close()

    # --- broadcast b_proj to [128, D] fp32 via K=1 fp32 matmuls (placed at
    # the phase transition: off the startup critical path, and the PE is
    # otherwise underfed here) ---
    with tc.tile_pool(name="pbias", bufs=1, space="PSUM") as pbias:
        for j, w_ in ((0, 512), (1, 256)):
            bias_ps = pbias.tile([P, 512], F32)
            nc.tensor.matmul(bias_ps[:, :w_], ones_row[:],
                             bproj_row[:, ds(j * 512, w_)],
                             start=True, stop=True)
            nc.vector.tensor_copy(bias_sb[:, ds(j * 512, w_)], bias_ps[:, :w_])
        # Transition filler: dep-free full-array matmuls into the scratch
        # bank bridge the PE idle while kv_sb evacuates (results unused;
        # downstream pools reopen the bank with start=True).
        warm_ps = pbias.tile([P, 512], F32)
        for i in range(4):
            nc.tensor.matmul(warm_ps[:], wqkvT[:, i, 0:P],
                             qT_all[:, i, 0:512],
                             start=True, stop=True, skip_group_check=True)

    zps = stk.enter_context(tc.tile_pool(name="zps", bufs=2, space="PSUM"))
    atps = stk.enter_context(tc.tile_pool(name="atps", bufs=2, space="PSUM"))
    ops_ = stk.enter_context(tc.tile_pool(name="ops", bufs=2, space="PSUM"))
    ph2 = stk.enter_context(tc.tile_pool(name="ph2", bufs=3))
    zrpool = stk.enter_context(tc.tile_pool(name="zr", bufs=4))

    # ============ PHASE 2: z, attn out, proj ==============================
    # Per chunk: z_pre = ksel.T @ qT (PE), z = recip(z_pre + eps) (ACT),
    # z replicated to head-dim partitions via a DRAM bounce + broadcast-read
    # DMAs (stride-0 DRAM source) -- no PE/DVE cost for the replication.
    # The attention matmul runs on the UNSCALED qT (z scaling commutes with
    # the per-head contraction) and the z factor is applied during the PSUM
    # evacuation, so the attn matmuls depend only on kv_sb + qT_all and give
    # the PE real work while the first zr tiles are in flight.
    def z_chain(c):
        qT = qT_all[:, :, ts(c, CW)]
        z_ps = zps.tile([H, CW], F32)
        for kt in range(KT):
            nc.tensor.matmul(z_ps[:], ksel[:, kt], qT[:, kt],
                             start=(kt == 0), stop=(kt == KT - 1))
        zb = ph2.tile([H, CW], BF16, tag="zb")
        nc.scalar.add_instruction(mybir.InstActivation(
            name=nc.get_next_instruction_name(),
            func=AF.Reciprocal,
            ins=[nc.scalar.lower_ap(z_ps[:]),
                 mybir.ImmediateValue(dtype=F32, value=EPS),
                 mybir.ImmediateValue(dtype=F32, value=1.0),
                 mybir.ImmediateValue(dtype=F32, value=0.0)],
            outs=[nc.scalar.lower_ap(zb[:])]))
        # bounce zb to DRAM, then broadcast-read z rows into [128, KT, CW]
        zb_dram = dramp.tile([H, CW], BF16)
        nc.sync.dma_start(zb_dram[:], zb[:])
        zr = zrpool.tile([P, KT, CW], BF16, tag="zr")
        for kt in range(KT):
            for s_ in range(2):
                h = 2 * kt + s_
                nc.sync.dma_start(
                    zr[ds(64 * s_, 64), kt],
                    zb_dram[h:h + 1, :].broadcast_to([64, CW]))
        return zr

    def attn_mms(c, fused):
        """attn_T[e, t] per head pair on unscaled qT; parity s in its own
        quadrant. fused=True: z-scale applied during the DVE evacuation
        (waits on zr). fused=False: plain ACT-copy evac (no zr dep) -- the
        caller scales in place later."""
        qT = qT_all[:, :, ts(c, CW)]
        attnT = ph2.tile([P, KT, CW], BF16, tag="attnT")
        for p_ in range(KT):
            at_ps = atps.tile([P, CW], F32)
            for s_ in range(2):
                nc.tensor.matmul(
                    at_ps[ds(64 * s_, 64), :],
                    kv_sb[ds(64 * s_, 64), ds(65 * p_, 64)],
                    qT[ds(64 * s_, 64), p_],
                    start=True, stop=True,
                    tile_position=(64 * s_, 64 * s_))
            if fused:
                nc.vector.tensor_mul(attnT[:, p_], at_ps[:],
                                     zr_tiles[c][:, p_])
            else:
                nc.scalar.activation(attnT[:, p_], at_ps[:], AF.Copy)
        return attnT

    # z three chunks ahead (the DRAM round trip needs ~8us of cover);
    # attn two chunks ahead (dep-free filler for the transition). The attn
    # matmuls need only kv_sb (ready before ksel), so they issue first.
    zr_tiles = {}
    attn_tiles = {}
    attn_tiles[0] = attn_mms(0, fused=False)
    attn_tiles[1] = attn_mms(1, fused=False)
    for c0 in range(3):
        zr_tiles[c0] = z_chain(c0)

    for c in range(CH):
        attnT = attn_tiles.pop(c)
        zr = zr_tiles.pop(c)
        if c < 2:
            # late in-place z-scale (zr was still in flight at issue time)
            for kt in range(KT):
                nc.vector.tensor_mul(attnT[:, kt], attnT[:, kt], zr[:, kt])
        if c == 0:
            dump("zr0", zr[:], [P, KT, CW], BF16)
            dump("attnT0", attnT[:], [P, KT, CW], BF16)

        # proj: out[t, e] = sum_d attnT[d, t] * wprojT[d, e]  (+ bias)
        for tt in range(TPC):
            t = c * TPC + tt
            o_ps = ops_.tile([P, D], F32)
            for kt in range(KT):
                for j, w_ in ((0, 512), (1, 256)):
                    mm = nc.tensor.matmul(
                        o_ps[:, ds(j * 512, w_)],
                        attnT[:, kt, ts(tt, P)],
                        wprojT[:, kt, ds(j * 512, w_)],
                        start=(kt == 0), stop=(kt == KT - 1))
                    if j > 0 and LDW_SKIP:
                        mm.ldweights = False  # same stationary as j-1
            osb = ph2.tile([P, D], BF16, tag="osb")
            nc.vector.tensor_tensor(osb[:], o_ps[:], bias_sb[:], ALU.add)
            nc.scalar.dma_start(out_d[ts(t, P), :], osb[:])

        if c + 3 < CH:
            zr_tiles[c + 3] = z_chain(c + 3)
        if c + 2 < CH:
            attn_tiles[c + 2] = attn_mms(c + 2, fused=True)


_CACHE = {}


def _get_nc(with_qkv_bias: bool, dbg: bool = False):
    key = ("nc", with_qkv_bias, dbg)
    if key not in _CACHE:
        _CACHE[key] = _build(with_qkv_bias, dbg)
    return _CACHE[key]


def kernel(x, ln_gamma, ln_beta, w_qkv, w_proj, b_proj, trace=False, dbg=False):
    x = np.asarray(x, dtype=np.float32)
    ln_gamma = np.asarray(ln_gamma, dtype=np.float32)
    ln_beta = np.asarray(ln_beta, dtype=np.float32)
    w_qkv = np.asarray(w_qkv, dtype=np.float32)
    w_proj = np.asarray(w_proj, dtype=np.float32)
    b_proj = np.asarray(b_proj, dtype=np.float32)
    bsz = x.shape[0]
    assert x.shape == (bsz, N, D) and bsz == N_CORES

    # Fold LN affine into the qkv projection (exact algebra):
    #   y = xhat*gamma + beta  =>  qkv = xhat @ (gamma*W)^T + W@beta
    wq_eff = (w_qkv * ln_gamma[None, :])          # [E3, D]
    cqkv = w_qkv @ ln_beta                        # [E3]
    with_bias = bool(np.any(cqkv))
    if with_bias:
        raise NotImplementedError(
            "nonzero W@beta path not wired into the device kernel")

    wqkvT = np.ascontiguousarray(wq_eff.T).astype(ml_dtypes.bfloat16)
    wprojT = np.ascontiguousarray(w_proj.T).astype(ml_dtypes.bfloat16)
    bpack = np.concatenate([np.ones(P, np.float32),
                            b_proj.astype(np.float32)]).reshape(1, P + D)

    # If the caller's process pinned jax to cpu (common for reference
    # generation), re-discover the neuron/axon backend before the PJRT run.
    import jax
    if len(jax.devices()) < N_CORES:
        try:
            jax.config.update("jax_platforms", None)
            jax.clear_backends()
        except Exception:
            pass

    nc = _get_nc(with_bias, dbg)
    in_maps = []
    for i in range(N_CORES):
        m = {"x": np.ascontiguousarray(x[i]).astype(ml_dtypes.bfloat16),
             "wqkvT": wqkvT, "wprojT": wprojT, "bpack": bpack}
        in_maps.append(m)

    res = run_bass_kernel_spmd(nc, in_maps, core_ids=list(range(N_CORES)),
                               trace=trace)
    out = np.stack([np.asarray(res.results[i]["out"]).astype(np.float32)
                    for i in range(N_CORES)], axis=0)
    if dbg:
        return out, res
    if trace:
        return out, res
    return out
